# revision 27
# baseline (speedup 1.0000x reference)
"""CAB multi-head attention on 8 Trainium2 NeuronCores.

Sharding: fully data-parallel, core c -> (batch b = c//2, query-half = c%2).
Each core computes 256 query rows against all 512 keys of its batch.
No collectives. Host does transposes/packing; device does all FLOPs.

Per-core layout conventions (features on partitions, tokens on free):
  QT/KT [E, t] f32r; V [s, e] f32r; scoresT/attnT [s, t] (softmax along
  partitions via one-hot-column matmuls, no max subtraction needed);
  CAB pairs i-major: h/h2 [(d, i%2), j]; comp [(iic, i%2+h), j] is
  PE-transposed into biasT [j, (tt, jc, c)] and pre-loaded into the
  scores PSUM via an identity matmul with a strided moving AP.

Engine plan (v2): the CAB pair-MLP stage is elementwise-bound, so its
per-tile ops are split evenly between the ACT engine (activation with
per-partition bias does relu(hjT + hi_col) in one pass) and the DVE.
GPSIMD is never used (its tensor_scalar is ~10x slower and its SBUF
traffic stalls the other engines). The Q/K/V projection matmuls are
interleaved through the CAB loop so the PE never idles long enough to
drop to the cold 1.2 GHz clock.
"""
import sys

sys.path.insert(0, "/opt/trn_rl_repo")

import numpy as np
import ml_dtypes
from contextlib import ExitStack

import concourse.bacc as bacc
import concourse.tile as tile
from concourse import mybir
from concourse.bass_utils import run_bass_kernel_spmd

F32 = mybir.dt.float32
F32R = mybir.dt.float32r
BF16 = mybir.dt.bfloat16
AF = mybir.ActivationFunctionType
ALU = mybir.AluOpType

B, N, E, H, SD, HID = 4, 512, 1024, 16, 64, 64
D = E // H
NQ = 256            # query rows per core
NCORES = 8
NTT = NQ // 8       # 32 tt groups (4 i-pairs each) in the CAB stage

_BF = ml_dtypes.bfloat16


def _build_program(debug=False):
    nc = bacc.Bacc("TRN2", target_bir_lowering=False, debug=False,
                   num_devices=NCORES)

    def din(name, shape, dt):
        return nc.dram_tensor(name, list(shape), dt, kind="ExternalInput").ap()

    d = {}
    d["qT"] = din("qT", (E, NQ), F32R)
    d["kT"] = din("kT", (E, N), F32R)
    d["vT"] = din("vT", (E, N), F32R)
    d["seT"] = din("seT", (SD, N), F32R)
    d["seQ"] = din("seQ", (SD, NQ), F32R)
    d["wq"] = din("wq", (E, E), F32R)
    d["wk"] = din("wk", (E, E), F32R)
    d["wv"] = din("wv", (E, E), F32R)
    d["wo"] = din("wo", (E, E), BF16)
    d["w1a"] = din("w1a", (SD, 128), F32R)
    d["w1b"] = din("w1b", (SD, 128), F32R)
    d["w2bd"] = din("w2bd", (128, 128), BF16)
    d["w3bd"] = din("w3bd", (128, 32), BF16)
    d["id128"] = din("id128", (128, 128), BF16)
    d["hsel"] = din("hsel", (128, 32), BF16)
    d["bq128"] = din("bq128", (128, 8), F32)
    d["bk128"] = din("bk128", (128, 8), F32)
    d["b1d"] = din("b1d", (128, 1), F32)
    d["b2d"] = din("b2d", (128, 1), F32)
    d["t128"] = din("t128", (128, 1), F32)
    d["b3t"] = din("b3t", (128, 1), F32)
    d["bv2d"] = din("bv2d", (1, E), F32R)
    d["ones1"] = din("ones1", (1, 128), F32R)
    out_d = nc.dram_tensor("out", [NQ, E], F32, kind="ExternalOutput").ap()
    rscratch = nc.dram_tensor("rscratch", [16, NQ], F32).ap()

    with tile.TileContext(nc) as tc, ExitStack() as ctx:
        # ---------------- persistent SBUF pools ----------------
        cst = ctx.enter_context(tc.tile_pool(name="cst", bufs=1))
        big = ctx.enter_context(tc.tile_pool(name="big", bufs=1))

        def cload(name, shape, dt):
            t = cst.tile(list(shape), dt, tag=name, name=name)
            nc.sync.dma_start(t[:], d[name][:])
            return t

        # critical-path loads first: W1 inputs unblock stage A, then the
        # CAB weights for the first tt iterations
        w1a = cload("w1a", (SD, 128), F32R)
        w1b = cload("w1b", (SD, 128), F32R)
        seT = cload("seT", (SD, N), F32R)
        seQ = cload("seQ", (SD, NQ), F32R)
        w2bd = cload("w2bd", (128, 128), BF16)
        b1d = cload("b1d", (128, 1), F32)
        b2d = cload("b2d", (128, 1), F32)
        id128 = cload("id128", (128, 128), BF16)
        w3bd = cload("w3bd", (128, 32), BF16)
        t128 = cload("t128", (128, 1), F32)
        b3t = cload("b3t", (128, 1), F32)
        bq128 = cload("bq128", (128, 8), F32)
        bk128 = cload("bk128", (128, 8), F32)
        bv2d = cload("bv2d", (1, E), F32R)
        ones1 = cload("ones1", (1, 128), F32R)
        hsel = cload("hsel", (128, 32), BF16)

        # resident per-core inputs, chunked on k (one DMA each, k-chunk kc
        # of a [E, t] tensor lives in tile kc as [128, t]).  Bulk weight
        # loads dispatch from the otherwise-idle gpsimd queue so the sync
        # engine's descriptor generation doesn't delay the critical loads.
        def kchunks(name, t, dt, ntile=8, eng=None):
            eng = eng or nc.sync
            ts = []
            for k in range(ntile):
                tt = big.tile([128, t], dt, tag=f"{name}{k}", name=f"{name}{k}")
                eng.dma_start(tt[:], d[name][k * 128:(k + 1) * 128, :])
                ts.append(tt)
            return ts

        qTt = kchunks("qT", NQ, F32R)
        kTt = kchunks("kT", N, F32R)
        # Wv rows resident (rhs of V-proj), Wo rows resident (rhs of out-proj)
        wv_r = kchunks("wv", E, F32R, eng=nc.gpsimd)
        wo_r = kchunks("wo", E, BF16, eng=nc.gpsimd)

        # persistent intermediates
        QT = [big.tile([128, NQ], F32R, tag=f"QT{k}", name=f"QT{k}") for k in range(8)]
        KT = [big.tile([128, N], F32R, tag=f"KT{k}", name=f"KT{k}") for k in range(8)]
        Vsb = [[big.tile([128, 512], BF16, tag=f"V{st}_{et}", name=f"V{st}_{et}")
                for et in range(2)] for st in range(4)]
        hjT = big.tile([128, N], BF16, tag="hjT")
        hiT = big.tile([128, 128], F32, tag="hiT")
        # biasT free layout: h*1024 + jc*256 + tt*8 + iic*2 + par (h-major so
        # the phase-C bias inject streams a dense [128, 256] block per (h, jc))
        biasT = big.tile([128, NTT * 512], BF16, tag="biasT")
        avN = [big.tile([128, NQ], BF16, tag=f"avN{hp}", name=f"avN{hp}") for hp in range(8)]

        # ---------------- stage A: W1 (tiny) ----------------
        with tc.tile_pool(name="w1ps", bufs=1, space="PSUM") as w1ps:
            hj_ps = w1ps.tile([128, N], F32, tag="hjps")
            nc.tensor.matmul(hj_ps[:], w1b[:], seT[:], start=True, stop=True)
            nc.scalar.activation(hjT[:], hj_ps[:], AF.Identity,
                                 bias=b1d[:, 0:1])
            hi_ps = w1ps.tile([128, NQ], F32, tag="hips")
            nc.tensor.matmul(hi_ps[:], w1a[:], seQ[:], start=True, stop=True)
            hi_v = hi_ps[:].rearrange("p (i two) -> p i two", two=2)
            nc.vector.tensor_copy(hiT[0:64, :], hi_v[0:64, :, 0])
            nc.vector.tensor_copy(hiT[64:128, :], hi_v[64:128, :, 1])

        # ---------------- stage B: CAB pair-MLP + QKV projections ----------
        with tc.tile_pool(name="wcol", bufs=4) as wcol, \
             tc.tile_pool(name="p1ps", bufs=2, space="PSUM") as p1ps, \
             tc.tile_pool(name="hpool", bufs=6) as hpool, \
             tc.tile_pool(name="h2sb", bufs=6) as h2sbp, \
             tc.tile_pool(name="csb", bufs=4) as csbp, \
             tc.tile_pool(name="h2ps", bufs=2, space="PSUM") as h2ps, \
             tc.tile_pool(name="cps", bufs=2, space="PSUM") as cps, \
             tc.tile_pool(name="trps", bufs=2, space="PSUM") as trps:

            # ---- projection jobs, interleaved through the tt loop ----
            # (head-chunks 5-7 of Q/K are only consumed by stage-C pairs 5-7,
            # so those jobs run inside stage C to keep its PE un-throttled)
            def q_job(ec, wpool, pspool):
                wq_c = wpool.tile([128, 1024], F32R, tag="wcol")
                nc.sync.dma_start(
                    wq_c[:],
                    d["wq"][:, ec * 128:(ec + 1) * 128]
                    .rearrange("(k p) c -> p k c", p=128))
                ps = pspool.tile([128, 512], F32, tag="p1", name="qps")[:, 0:NQ]
                for kc in range(8):
                    nc.tensor.matmul(ps[:], wq_c[:, kc * 128:(kc + 1) * 128],
                                     qTt[kc][:], start=(kc == 0),
                                     stop=(kc == 7))
                if ec % 2 == 0:
                    nc.scalar.activation(QT[ec][:], ps[:],
                                         AF.Identity, bias=bq128[:, ec:ec + 1])
                else:
                    nc.vector.tensor_scalar(QT[ec][:], ps[:],
                                            bq128[:, ec:ec + 1], None, ALU.add)

            def k_job(ec, wpool, pspool):
                wk_c = wpool.tile([128, 1024], F32R, tag="wcol")
                nc.sync.dma_start(
                    wk_c[:],
                    d["wk"][:, ec * 128:(ec + 1) * 128]
                    .rearrange("(k p) c -> p k c", p=128))
                ps = pspool.tile([128, 512], F32, tag="p1", name="kvps")
                for kc in range(8):
                    nc.tensor.matmul(ps[:], wk_c[:, kc * 128:(kc + 1) * 128],
                                     kTt[kc][:], start=(kc == 0),
                                     stop=(kc == 7))
                if ec % 2 == 0:
                    nc.scalar.activation(KT[ec][:], ps[:],
                                         AF.Identity, bias=bk128[:, ec:ec + 1])
                else:
                    nc.vector.tensor_scalar(KT[ec][:], ps[:],
                                            bk128[:, ec:ec + 1], None, ALU.add)

            def v_job(st):
                # et=0 half only; the et=1 half runs inside stage C (its
                # long full-width matmuls keep the HAM un-throttled there)
                vt_c = wcol.tile([128, 1024], F32R, tag="wcol")
                nc.sync.dma_start(
                    vt_c[:],
                    d["vT"][:, st * 128:(st + 1) * 128]
                    .rearrange("(k p) c -> p k c", p=128))
                ps = p1ps.tile([128, 512], F32, tag="p1", name="kvps")
                for kc in range(8):
                    nc.tensor.matmul(
                        ps[:], vt_c[:, kc * 128:(kc + 1) * 128],
                        wv_r[kc][:, 0:512],
                        start=(kc == 0), stop=False)
                nc.tensor.matmul(ps[:], ones1[0:1, 0:128],
                                 bv2d[0:1, 0:512],
                                 start=False, stop=True)
                nc.vector.tensor_copy(Vsb[st][0][:], ps[:])

            jobs = ([lambda ec=ec: k_job(ec, wcol, p1ps) for ec in range(5)]
                    + [lambda ec=ec: q_job(ec, wcol, p1ps) for ec in range(5)]
                    + [lambda st=st: v_job(st) for st in range(4)])
            njobs = len(jobs)
            job_i = 0

            for tt in range(NTT):
                # spread the 20 projection jobs evenly over 32 tt iterations
                while job_i < njobs and job_i < (tt + 1) * njobs // NTT:
                    jobs[job_i]()
                    job_i += 1

                h2_tiles = []
                for iic in range(4):
                    ii = tt * 4 + iic
                    h_t = hpool.tile([128, N], BF16, tag="h")
                    if iic % 2 == 0:
                        nc.scalar.activation(h_t[:], hjT[:], AF.Relu,
                                             bias=hiT[:, ii:ii + 1])
                    else:
                        nc.vector.tensor_scalar(h_t[:], hjT[:],
                                                hiT[:, ii:ii + 1], 0.0,
                                                ALU.add, ALU.max)
                    ps = h2ps.tile([128, N], F32, tag="h2")
                    nc.tensor.matmul(ps[:], w2bd[:], h_t[:], start=True,
                                     stop=True)
                    h2_t = h2sbp.tile([128, N], BF16, tag="h2sb")
                    if iic % 2 == 0:
                        nc.vector.tensor_scalar(h2_t[:], ps[:], b2d[:, 0:1],
                                                0.0, ALU.add, ALU.max)
                    else:
                        nc.scalar.activation(h2_t[:], ps[:], AF.Relu,
                                             bias=b2d[:, 0:1])
                    h2_tiles.append(h2_t)

                c_ps = cps.tile([128, N], F32, tag="comp")
                for iic in range(4):
                    nc.tensor.matmul(c_ps[32 * iic:32 * iic + 32, :],
                                     w3bd[:], h2_tiles[iic][:],
                                     start=True, stop=True,
                                     tile_position=(0, 32 * iic))
                c_sb = csbp.tile([128, N], BF16, tag="csb")
                if tt % 2 == 0:
                    nc.vector.tensor_scalar(c_sb[:], c_ps[:], t128[:, 0:1],
                                            b3t[:, 0:1], ALU.mult, ALU.add)
                else:
                    nc.scalar.activation(c_sb[:], c_ps[:], AF.Identity,
                                         bias=b3t[:, 0:1], scale=t128[:, 0:1])
                tr_ps = trps.tile([128, 512], BF16, tag="tr")
                for jc in range(4):
                    nc.tensor.transpose(tr_ps[:, jc * 128:(jc + 1) * 128],
                                        c_sb[:, jc * 128:(jc + 1) * 128],
                                        id128[:])
                # scatter tr_ps [j, (iic, h, par)] into the h-major biasT
                bT6 = biasT[:].rearrange("p (x j t i m) -> p j t i x m",
                                         x=16, j=4, t=NTT, i=4, m=2)
                for jc in range(4):
                    src = tr_ps[:, jc * 128:(jc + 1) * 128].rearrange(
                        "p (i x m) -> p i x m", i=4, x=16, m=2)
                    nc.vector.tensor_copy(bT6[:, jc, tt], src)

        # ---------------- stage C: scores + softmax + AV ----------------
        # Half-head score tiles ([128, 512] = 2 key-chunks) at bufs=4 keep
        # the PE 2 halves ahead of the exp, so HAM stays warm.  Softmax
        # denominators are accumulated per head-pair and the reciprocal /
        # broadcast / normalize chain runs inline so nothing serializes at
        # the end of the stage.
        with tc.tile_pool(name="attnT", bufs=4) as attp, \
             tc.tile_pool(name="vcol", bufs=2) as vcol, \
             tc.tile_pool(name="scps", bufs=4, space="PSUM") as scps, \
             tc.tile_pool(name="smps", bufs=1, space="PSUM") as smps, \
             tc.tile_pool(name="avps", bufs=2, space="PSUM") as avps, \
             tc.tile_pool(name="vps", bufs=1, space="PSUM") as vps, \
             tc.tile_pool(name="r2sb", bufs=2) as r2sb, \
             tc.tile_pool(name="rc2", bufs=2) as rc2p:

            def v_job_c(st):
                vt_c = vcol.tile([128, 1024], F32R, tag="vcol")
                nc.sync.dma_start(
                    vt_c[:],
                    d["vT"][:, st * 128:(st + 1) * 128]
                    .rearrange("(k p) c -> p k c", p=128))
                ps = vps.tile([128, 512], F32, tag="p1")
                for kc in range(8):
                    nc.tensor.matmul(
                        ps[:], vt_c[:, kc * 128:(kc + 1) * 128],
                        wv_r[kc][:, 512:1024],
                        start=(kc == 0), stop=False)
                nc.tensor.matmul(ps[:], ones1[0:1, 0:128],
                                 bv2d[0:1, 512:1024],
                                 start=False, stop=True)
                nc.scalar.copy(Vsb[st][1][:], ps[:])

            # per-pair long-matmul jobs: V et=1 chunks + the Q/K projection
            # chunks for heads 10-15 (each needed only from its own pair on)
            cjobs = {
                0: [lambda: v_job_c(0), lambda: k_job(5, vcol, vps)],
                1: [lambda: v_job_c(1), lambda: q_job(5, vcol, vps)],
                2: [lambda: v_job_c(2), lambda: k_job(6, vcol, vps)],
                3: [lambda: v_job_c(3), lambda: q_job(6, vcol, vps)],
                4: [lambda: k_job(7, vcol, vps)],
                5: [lambda: q_job(7, vcol, vps)],
            }

            def warm_mm():
                # scratch full-width matmul: keeps the HAM activity monitor
                # above its un-throttle threshold on job-less pairs
                wp = vps.tile([128, 512], F32, tag="p1", name="warm")
                nc.tensor.matmul(wp[:], id128[:], hjT[:], start=True,
                                 stop=True)

            bTh = biasT[:].rearrange("p (x r) -> p x r", x=16)
            for h in range(16):
                hp, hw = h // 2, (h % 2) * 64
                if h % 2 == 0:
                    for job in cjobs.get(hp, []):
                        job()
                    if hp >= 6:
                        warm_mm()
                    av_ps = avps.tile([128, NQ], F32, tag="av")
                    sums2 = smps.tile([2, NQ], F32, tag="s2")
                ats = []
                for half in range(2):
                    sc_ps = scps.tile([128, 512], F32, tag="sc")
                    for q in range(2):
                        jc = half * 2 + q
                        nc.tensor.matmul(
                            sc_ps[:, q * 256:(q + 1) * 256],
                            KT[hp][hw:hw + 64, jc * 128:(jc + 1) * 128],
                            QT[hp][hw:hw + 64, :],
                            start=True, stop=True, skip_group_check=True)
                    # bias add on the DVE (PE stays free for matmul rows)
                    nc.vector.tensor_tensor(
                        sc_ps[:], sc_ps[:],
                        bTh[:, h, half * 512:(half + 1) * 512], ALU.add)
                    at = attp.tile([128, 512], BF16, tag="at")
                    nc.scalar.activation(at[:], sc_ps[:], AF.Exp)
                    ats.append(at)
                for jc in range(4):
                    atv = ats[jc // 2][:, (jc % 2) * 256:(jc % 2 + 1) * 256]
                    nc.tensor.matmul(
                        sums2[:], hsel[:, 2 * h:2 * h + 2], atv,
                        start=(h % 2 == 0 and jc == 0),
                        stop=(h % 2 == 1 and jc == 3), skip_group_check=True)
                for jc in range(4):
                    st, et = jc, h // 8
                    atv = ats[jc // 2][:, (jc % 2) * 256:(jc % 2 + 1) * 256]
                    nc.tensor.matmul(
                        av_ps[hw:hw + 64, :],
                        Vsb[st][et][:, (h % 8) * 64:(h % 8) * 64 + 64],
                        atv,
                        start=(jc == 0), stop=(jc == 3),
                        skip_group_check=True,
                        tile_position=(0, hw))
                if h % 2 == 1:
                    recip2 = rc2p.tile([2, NQ], F32, tag="rc2")
                    nc.vector.reciprocal_approx_fast(recip2[:], sums2[:])
                    nc.sync.dma_start(rscratch[2 * hp:2 * hp + 2, :],
                                      recip2[:])
                    r2 = r2sb.tile([128, NQ], F32, tag="r2")
                    rsrc = rscratch[2 * hp:2 * hp + 2, :].rearrange(
                        "h (o t) -> h o t", o=1)
                    nc.sync.dma_start(r2[:], rsrc.broadcast_to([2, 64, NQ]))
                    nc.vector.tensor_tensor(avN[hp][:], av_ps[:], r2[:],
                                            ALU.mult)

        # ---------------- stage D: output projection ----------------
        with tc.tile_pool(name="osb", bufs=2) as osb, \
             tc.tile_pool(name="ops", bufs=2, space="PSUM") as ops:
            for ttile in range(2):
                for et in range(2):
                    ps = ops.tile([128, 512], F32, tag="ops")
                    for hp in range(8):
                        nc.tensor.matmul(
                            ps[:], avN[hp][:, ttile * 128:(ttile + 1) * 128],
                            wo_r[hp][:, et * 512:(et + 1) * 512],
                            start=(hp == 0), stop=(hp == 7))
                    o_sb = osb.tile([128, 512], F32, tag="osb")
                    nc.scalar.copy(o_sb[:], ps[:])
                    nc.sync.dma_start(
                        out_d[ttile * 128:(ttile + 1) * 128,
                              et * 512:(et + 1) * 512], o_sb[:])

    nc.compile()
    return nc


def _host_prep(inputs):
    """Build the 8 per-core input maps from the full inputs."""
    f32 = np.float32
    q = np.ascontiguousarray(inputs["query"], f32)
    k = np.ascontiguousarray(inputs["key"], f32)
    v = np.ascontiguousarray(inputs["value"], f32)
    se = np.ascontiguousarray(inputs["state_embeddings"], f32)
    scale = f32(D) ** f32(-0.5)
    wq = np.ascontiguousarray(inputs["Wq"] * scale, f32)
    wk = np.ascontiguousarray(inputs["Wk"], f32)
    wv = np.ascontiguousarray(inputs["Wv"], f32)
    wo = np.ascontiguousarray(inputs["Wo"]).astype(_BF)
    bq = np.asarray(inputs["bq"], f32) * scale
    bk = np.asarray(inputs["bk"], f32)
    bv = np.asarray(inputs["bv"], f32)
    w1 = np.asarray(inputs["W1"], f32)
    b1 = np.asarray(inputs["b1"], f32)
    w2 = np.asarray(inputs["W2"], f32)
    b2 = np.asarray(inputs["b2"], f32)
    w3 = np.asarray(inputs["W3"], f32)
    b3 = np.asarray(inputs["b3"], f32)
    temps = np.asarray(inputs["head_temps"], f32)

    w1a_dup = np.concatenate([w1[:SD], w1[:SD]], axis=1)          # [64,128]
    w1b_dup = np.concatenate([w1[SD:], w1[SD:]], axis=1)          # [64,128]
    w2bd = np.zeros((128, 128), f32)
    w2bd[:64, :64] = w2
    w2bd[64:, 64:] = w2
    w3bd = np.zeros((128, 32), f32)
    w3bd[:64, 0::2] = w3         # m = 2*h + par (h-major pairs)
    w3bd[64:, 1::2] = w3
    hsel = np.zeros((128, 32), f32)
    for h in range(H):
        hsel[:, 2 * h + h % 2] = 1.0
    hidx = (np.arange(128) % 32) // 2
    t128 = temps[hidx].reshape(128, 1)
    b3t = (b3 * temps)[hidx].reshape(128, 1)
    b1d = np.tile(b1, 2).reshape(128, 1)
    b2d = np.tile(b2, 2).reshape(128, 1)
    bq128 = bq.reshape(8, 128).T.copy()
    bk128 = bk.reshape(8, 128).T.copy()
    id128 = np.eye(128, dtype=f32).astype(_BF)
    ones1 = np.ones((1, 128), f32)
    bv2d = bv.reshape(1, E)

    shared = dict(wq=wq, wk=wk, wv=wv, wo=wo, w1a=w1a_dup, w1b=w1b_dup,
                  w2bd=w2bd.astype(_BF), w3bd=w3bd.astype(_BF),
                  id128=id128, hsel=hsel.astype(_BF), bq128=bq128, bk128=bk128,
                  b1d=b1d, b2d=b2d, t128=t128, b3t=b3t, bv2d=bv2d,
                  ones1=ones1)
    maps = []
    for c in range(NCORES):
        b, half = c // 2, c % 2
        rows = slice(half * NQ, (half + 1) * NQ)
        m = dict(shared)
        m["qT"] = np.ascontiguousarray(q[b, rows].T)
        m["kT"] = np.ascontiguousarray(k[b].T)
        m["vT"] = np.ascontiguousarray(v[b].T)
        m["seT"] = np.ascontiguousarray(se[b].T)
        m["seQ"] = np.ascontiguousarray(se[b, rows].T)
        maps.append(m)
    return maps


_cache = {}


def _get_program():
    if "nc" not in _cache:
        _cache["nc"] = _build_program()
    return _cache["nc"]


def kernel(**inputs):
    nc = _get_program()
    maps = _host_prep(inputs)
    res = run_bass_kernel_spmd(nc, maps, list(range(NCORES)))
    bo = np.asarray(inputs["bo"], np.float32)
    out = np.empty((B, N, E), np.float32)
    for c in range(NCORES):
        b, half = c // 2, c % 2
        out[b, half * NQ:(half + 1) * NQ] = res.results[c]["out"]
    return out + bo
